# revision 27
# baseline (speedup 1.0000x reference)
"""ChronoFormer Trainium2 kernel.

Sharding: batch-parallel, core pairs (2b, 2b+1) redundantly compute batch b
(no collectives — a pairwise AllGather measured 94us of dead link time, more
than the compute it saved). Host side does indexing only: per batch the
sequence is permuted to [orig pos 2047, unmasked keys..., masked...] and
truncated to KEPT=1280 slots. Masked keys contribute exp(-1e9)=0 via the
per-key bias, and layer outputs are only ever read at unmasked slots +
slot 0 (layer 2 needs keys at unmasked slots and the single query at orig
pos 2047), so the whole model runs on the KEPT range.

On-device: activations transposed (feature dim on partitions), bf16 matmul
operands with fp32 PSUM accumulation. Scores are computed transposed (keys
on partitions) so the per-key time-bias + mask and the 1/sqrt(dk) scale fold
into the scalar-engine exp (out = exp(scale*in + bias)); the two 512-wide
query chunks of one (key-tile, head) share that bias, so their exp runs as
one strided ACT call over a 2-bank PSUM tile. Softmax stays unnormalized
through attn@V via a ones-column augmentation of V (row 64 of the ctx PSUM
accumulates the denominator); normalization is applied to the small ctx.
LayerNorm: ones-vector matmuls (fp32r) for partition sums, inv-std as
exp(-0.5*ln(var+eps)) — the whole kernel stays on one ACT table set
(natural_log_exp_and_others); the final sigmoid is exp + DVE reciprocal.
"""

import numpy as np

B, S, D, H, DK, LAYERS = 4, 2048, 256, 4, 64, 2
V, T = 32000, 1000
KEPT = 1280
KT = KEPT // 128          # 10 key tiles
PCH = [(0, 512), (512, 512), (1024, 256)]  # kept-range free-dim chunks
N_CORES = 8
EPS = 1e-5
SCALE = 1.0 / np.sqrt(DK).astype(np.float32)
NEG = -1e9

_CACHE = {}


def _build():
    import concourse.bass as bass
    import concourse.mybir as mybir
    import concourse.tile as tile
    from concourse import bacc

    f32 = mybir.dt.float32
    F32R = mybir.dt.float32r
    BF16 = mybir.dt.bfloat16
    ACT = mybir.ActivationFunctionType
    ALU = mybir.AluOpType

    nc = bacc.Bacc("TRN2", target_bir_lowering=False, debug=True,
                   num_devices=N_CORES)

    he_d = nc.dram_tensor("he", [128, 2, KEPT], BF16, kind="ExternalInput")
    ht_d = nc.dram_tensor("ht", [128, 2, KEPT], BF16, kind="ExternalInput")
    b1_d = nc.dram_tensor("b1", [128, H * KT], f32, kind="ExternalInput")
    b2_d = nc.dram_tensor("b2", [128, H * KT], f32, kind="ExternalInput")
    wall_d = nc.dram_tensor("wall", [128, LAYERS * 4 * 2 * D], BF16,
                            kind="ExternalInput")
    ball_d = nc.dram_tensor("ball", [128, LAYERS * 4 * 2], f32,
                            kind="ExternalInput")
    bvrow_d = nc.dram_tensor("bvrow", [1, LAYERS, D], f32, kind="ExternalInput")
    lngb_d = nc.dram_tensor("lngb", [128, LAYERS * 2 * 2], f32,
                            kind="ExternalInput")
    wc1_d = nc.dram_tensor("wc1", [128, 2 * 128], BF16, kind="ExternalInput")
    bc1_d = nc.dram_tensor("bc1", [128, 1], f32, kind="ExternalInput")
    wc2_d = nc.dram_tensor("wc2", [128, 1], BF16, kind="ExternalInput")
    bc2_d = nc.dram_tensor("bc2", [1, 1], f32, kind="ExternalInput")
    onec_d = nc.dram_tensor("onec", [128, 1], F32R, kind="ExternalInput")
    oneb_d = nc.dram_tensor("oneb", [128, 1], BF16, kind="ExternalInput")
    out_d = nc.dram_tensor("out", [1, 1], f32, kind="ExternalOutput")

    def r(ap):
        return ap.bitcast(F32R)

    with tile.TileContext(nc) as tc:
        with (
            tc.tile_pool(name="const", bufs=1) as cp,
            tc.tile_pool(name="work", bufs=1) as wp,
            tc.tile_pool(name="exp", bufs=4) as ep,
            tc.tile_pool(name="tmp", bufs=2) as tp,
            tc.tile_pool(name="rows", bufs=1) as rp,
        ):
            # ---- constant / input loads ----
            wall = cp.tile([128, LAYERS * 4 * 2 * D], BF16, tag="wall")
            nc.sync.dma_start(wall[:], wall_d[:])
            ball = cp.tile([128, LAYERS * 4 * 2], f32, tag="ball")
            nc.sync.dma_start(ball[:], ball_d[:])
            bvrow = cp.tile([1, LAYERS, D], f32, tag="bvrow")
            nc.sync.dma_start(bvrow[:], bvrow_d[:])
            lngb = cp.tile([128, LAYERS * 2 * 2], f32, tag="lngb")
            nc.sync.dma_start(lngb[:], lngb_d[:])
            b1 = cp.tile([128, H * KT], f32, tag="b1")
            nc.sync.dma_start(b1[:], b1_d[:])
            b2 = cp.tile([128, H * KT], f32, tag="b2")
            nc.sync.dma_start(b2[:], b2_d[:])
            wc1 = cp.tile([128, 2 * 128], BF16, tag="wc1")
            nc.sync.dma_start(wc1[:], wc1_d[:])
            bc1 = cp.tile([128, 1], f32, tag="bc1")
            nc.sync.dma_start(bc1[:], bc1_d[:])
            wc2 = cp.tile([128, 1], BF16, tag="wc2")
            nc.sync.dma_start(wc2[:], wc2_d[:])
            bc2 = cp.tile([1, 1], f32, tag="bc2")
            nc.sync.dma_start(bc2[:], bc2_d[:])
            ones_col = cp.tile([128, 1], F32R, tag="ones")
            nc.sync.dma_start(ones_col[:], onec_d[:])
            oneb = cp.tile([128, 1], BF16, tag="oneb")
            nc.sync.dma_start(oneb[:], oneb_d[:])
            eps_t = cp.tile([1, 1], f32, tag="eps")
            nc.vector.memset(eps_t[:], EPS)

            # PE warm-up burst + early ACT table load: runs during the input
            # DMA wait so HAM un-throttles and the exp table set is resident
            # before the first real matmul/activation.
            if True:  # WARMUP_DISABLED
                wout = cp.tile([1, 1], f32, tag="wout")
                nc.scalar.activation(wout[:], eps_t[:], ACT.Exp)

            he = wp.tile([128, 2, KEPT], BF16, tag="he")
            nc.sync.dma_start(he[:], he_d[:])
            ht = wp.tile([128, 2, KEPT], BF16, tag="ht")
            nc.sync.dma_start(ht[:], ht_d[:])

            def Wl(l, p, kc, mc):
                base = (((l * 4 + p) * 2 + kc) * D) + mc * 128
                return wall[:, base:base + 128]

            def Wfull(l, p, kc):
                base = ((l * 4 + p) * 2 + kc) * D
                return wall[:, base:base + D]

            def bl(l, p, mc):
                c = (l * 4 + p) * 2 + mc
                return ball[:, c:c + 1]

            def gb(l, g, kc):
                c = (l * 2 + g) * 2 + kc
                return lngb[:, c:c + 1]

            # embedding add (rounds to bf16 on write)
            h0 = wp.tile([128, 2, KEPT], BF16, tag="h0")
            nc.vector.tensor_tensor(out=h0[:], in0=he[:], in1=ht[:],
                                    op=ALU.add)

            def proj_T(l, p, rhs, chunks, out_sb, psum_pool, ptag):
                # transposed-output projection: out[dout, s] over given chunks
                for mc in range(2):
                    for (off, n) in chunks:
                        ps = psum_pool.tile([128, n], f32, tag=ptag)
                        for kc in range(2):
                            nc.tensor.matmul(
                                ps[:], Wl(l, p, kc, mc),
                                rhs[:, kc, off:off + n],
                                start=(kc == 0), stop=(kc == 1))
                        nc.vector.tensor_scalar(
                            out=out_sb[:, mc, off:off + n], in0=ps[:],
                            scalar1=bl(l, p, mc), scalar2=None, op0=ALU.add)

            def proj_V(l, rhs, out_sb, bvb, expb, psum_pool, ptag):
                # natural-output V projection into [s_tile, h, 0:64], rows
                # scaled by exp(key bias) (multiplicative softmax bias+mask);
                # col 64 = exp(key bias) itself (softmax denominator column).
                for st in range(KT):
                    ps = psum_pool.tile([128, D], f32, tag=ptag)
                    for kc in range(2):
                        nc.tensor.matmul(
                            ps[:], rhs[:, kc, st * 128:(st + 1) * 128],
                            Wfull(l, 2, kc),
                            start=(kc == 0), stop=(kc == 1))
                    for h in range(H):
                        eb = expb[:, h * KT + st:h * KT + st + 1]
                        tv = tp.tile([128, 64], f32, tag="tv")
                        nc.vector.tensor_tensor(
                            out=tv[:], in0=ps[:, h * 64:(h + 1) * 64],
                            in1=bvb[:, h * 64:(h + 1) * 64], op=ALU.add)
                        nc.vector.tensor_scalar(
                            out=out_sb[:, st, h, 0:64], in0=tv[:],
                            scalar1=eb, scalar2=None, op0=ALU.mult)
                        nc.vector.tensor_copy(out_sb[:, st, h, 64:65], eb)

            def layer_norm_T(l, ha, sq, out_sb, ncols, chunks, stat_pool):
                # stats + apply; ha/sq [128, 2, ncols] f32r SBUF
                m_row = rp.tile([1, ncols], f32, tag=f"m{l}")
                v_row = rp.tile([1, ncols], f32, tag=f"v{l}")
                for (off, n) in chunks:
                    ssum = stat_pool.tile([1, n], f32, tag="sts")
                    ssq = stat_pool.tile([1, n], f32, tag="stq")
                    g_ = r if n >= 256 else (lambda a: a.bitcast(f32))
                    for kc in range(2):
                        nc.tensor.matmul(ssum[:], g_(ones_col[:]),
                                         g_(ha[:, kc, off:off + n]),
                                         start=(kc == 0), stop=(kc == 1))
                        nc.tensor.matmul(ssq[:], g_(ones_col[:]),
                                         g_(sq[:, kc, off:off + n]),
                                         start=(kc == 0), stop=(kc == 1))
                    nc.vector.tensor_scalar(out=m_row[:, off:off + n],
                                            in0=ssum[:], scalar1=1.0 / D,
                                            scalar2=None, op0=ALU.mult)
                    nc.vector.tensor_scalar(out=v_row[:, off:off + n],
                                            in0=ssq[:], scalar1=1.0 / D,
                                            scalar2=None, op0=ALU.mult)
                msq = rp.tile([1, ncols], f32, tag=f"msq{l}")
                nc.vector.tensor_tensor(out=msq[:], in0=m_row[:], in1=m_row[:],
                                        op=ALU.mult)
                nc.vector.tensor_tensor(out=v_row[:], in0=v_row[:], in1=msq[:],
                                        op=ALU.subtract)
                vln = rp.tile([1, ncols], f32, tag=f"vln{l}")
                nc.scalar.activation(vln[:], v_row[:], ACT.Ln,
                                     bias=eps_t[0:1, 0:1])
                inv_row = rp.tile([1, ncols], f32, tag=f"inv{l}")
                nc.scalar.activation(inv_row[:], vln[:], ACT.Exp, scale=-0.5)
                m2_row = rp.tile([1, ncols], f32, tag=f"m2{l}")
                nc.vector.scalar_tensor_tensor(
                    out=m2_row[:], in0=m_row[:], scalar=-1.0, in1=inv_row[:],
                    op0=ALU.mult, op1=ALU.mult)
                invb = tp.tile([128, ncols], f32, tag="invb")
                nc.gpsimd.partition_broadcast(invb[:], inv_row[:])
                m2b = tp.tile([128, ncols], f32, tag="m2b")
                nc.gpsimd.partition_broadcast(m2b[:], m2_row[:])
                for kc in range(2):
                    for (off, n) in chunks:
                        t1 = tp.tile([128, n], f32, tag="t1")
                        nc.vector.tensor_tensor(
                            out=t1[:], in0=ha[:, kc, off:off + n],
                            in1=invb[:, off:off + n], op=ALU.mult)
                        t2 = tp.tile([128, n], f32, tag="t2")
                        nc.vector.tensor_tensor(
                            out=t2[:], in0=t1[:], in1=m2b[:, off:off + n],
                            op=ALU.add)
                        nc.vector.tensor_scalar(
                            out=out_sb[:, kc, off:off + n], in0=t2[:],
                            scalar1=gb(l, 0, kc), scalar2=gb(l, 1, kc),
                            op0=ALU.mult, op1=ALU.add)

            def attn_layer(qT, kT, vN, ctxT, hp):
                # scoresT -> exp (no bias: folded into V) -> ctx+denominator.
                # Main pass: query chunks (0,512),(512,512) in one 2-bank
                # PSUM tile per (kt, head), one strided exp call; the two
                # heads of chunk hp are emitted adjacently as row-groups
                # (0,*) / (64,*) so their score matmuls run concurrently.
                mc = hp
                pool = tc.tile_pool(name=f"psA{hp}", bufs=1, space="PSUM")
                psum_pool = pool.__enter__()
                ctx_ps = {}
                for hh in range(2):
                    for j in range(2):
                        ctx_ps[(hh, j)] = psum_pool.tile(
                            [65, 512], f32, name=f"ctx{hh}{j}",
                            tag=f"ctx{hh}{j}", bufs=1)
                for kt in range(KT):
                    s_t = {}
                    for hh in range(2):
                        s_t[hh] = psum_pool.tile([128, 1024], f32, tag="s",
                                                 name="s", bufs=2)
                    for j in range(2):
                        for hh in range(2):
                            hr = slice(hh * 64, hh * 64 + 64)
                            nc.tensor.matmul(
                                s_t[hh][:, j * 512:(j + 1) * 512],
                                kT[hr, mc, kt * 128:(kt + 1) * 128],
                                qT[hr, mc, j * 512:(j + 1) * 512],
                                start=True, stop=True,
                                tile_position=(hh * 64, 0))
                    for hh in range(2):
                        h = hp * 2 + hh
                        e_sb = ep.tile([128, 2, 512], BF16, tag="e")
                        nc.scalar.activation(
                            e_sb[:],
                            s_t[hh][:].rearrange("p (c q) -> p c q", c=2),
                            ACT.Exp, scale=float(SCALE))
                        for j in range(2):
                            nc.tensor.matmul(
                                ctx_ps[(hh, j)][0:65, :],
                                vN[:, kt, h, :], e_sb[:, j, :],
                                start=(kt == 0), stop=(kt == KT - 1))
                for hh in range(2):
                    for j in range(2):
                        _ctx_norm(ctx_ps[(hh, j)], ctxT, hh, hp, j * 512, 512)
                pool.__exit__(None, None, None)
                # tail pass: queries 1024:1280, both heads in one PSUM bank
                pool = tc.tile_pool(name=f"psAt{hp}", bufs=1, space="PSUM")
                tail_pool = pool.__enter__()
                ctx_tl = {}
                for hh in range(2):
                    ctx_tl[hh] = tail_pool.tile([65, 256], f32,
                                                name=f"ctxt{hh}",
                                                tag=f"ctxt{hh}", bufs=1)
                for kt in range(KT):
                    s_t = {}
                    for hh in range(2):
                        s_t[hh] = tail_pool.tile([128, 256], f32,
                                                 name="st", tag=f"st{hh}",
                                                 bufs=2)
                        hr = slice(hh * 64, hh * 64 + 64)
                        nc.tensor.matmul(
                            s_t[hh][:],
                            kT[hr, mc, kt * 128:(kt + 1) * 128],
                            qT[hr, mc, 1024:1280],
                            start=True, stop=True,
                            tile_position=(hh * 64, 0))
                    for hh in range(2):
                        h = hp * 2 + hh
                        e_sb = ep.tile([128, 256], BF16, tag="et")
                        nc.scalar.activation(e_sb[:], s_t[hh][:], ACT.Exp,
                                             scale=float(SCALE))
                        nc.tensor.matmul(
                            ctx_tl[hh][0:65, :],
                            vN[:, kt, h, :], e_sb[:],
                            start=(kt == 0), stop=(kt == KT - 1))
                for hh in range(2):
                    _ctx_norm(ctx_tl[hh], ctxT, hh, hp, 1024, 256)
                pool.__exit__(None, None, None)

            def _ctx_norm(ctx_ps, ctxT, hh, hp, qo, qn):
                r_sb = rp.tile([1, qn], f32, tag="r1", bufs=2, name="r_sb")
                nc.vector.reciprocal(r_sb[:], ctx_ps[64:65, :])
                rb = tp.tile([64, qn], f32, tag="rb", name="rb")
                nc.gpsimd.partition_broadcast(rb[:], r_sb[:])
                nc.vector.tensor_tensor(
                    out=ctxT[hh * 64:hh * 64 + 64, hp, qo:qo + qn],
                    in0=ctx_ps[0:64, :], in1=rb[:], op=ALU.mult)

            # ================= LAYER 1 =================
            kT1 = wp.tile([128, 2, KEPT], BF16, tag="kT1")
            qT1 = wp.tile([128, 2, KEPT], BF16, tag="qT1")
            vN1 = wp.tile([128, KT, H, 65], BF16, tag="vN1")
            expb1 = wp.tile([128, H * KT], f32, tag="expb1")
            nc.scalar.activation(expb1[:], b1[:], ACT.Exp)
            bvb1 = wp.tile([128, D], f32, tag="bvb1")
            nc.gpsimd.partition_broadcast(bvb1[:], bvrow[0:1, 0, :])

            with tc.tile_pool(name="psP1", bufs=2, space="PSUM") as pp1:
                proj_T(0, 1, h0, PCH, kT1, pp1, "pk")
                proj_T(0, 0, h0, PCH, qT1, pp1, "pq")
                proj_V(0, h0, vN1, bvb1, expb1, pp1, "pv")

            ctxT1 = wp.tile([128, 2, KEPT], BF16, tag="ctxT1")
            for hp in range(2):
                attn_layer(qT1, kT1, vN1, ctxT1, hp)

            ha1 = wp.tile([128, 2, KEPT], F32R, tag="ha1")
            sq1 = wp.tile([128, 2, KEPT], F32R, tag="sq1")
            h1 = wp.tile([128, 2, KEPT], BF16, tag="h1")
            with tc.tile_pool(name="psP3", bufs=2, space="PSUM") as pp3:
                for mc in range(2):
                    for (qoff, qn) in PCH:
                        ps = pp3.tile([128, qn], f32, tag="wo")
                        for kc in range(2):
                            nc.tensor.matmul(ps[:], Wl(0, 3, kc, mc),
                                             ctxT1[:, kc, qoff:qoff + qn],
                                             start=(kc == 0), stop=(kc == 1))
                        nc.vector.tensor_scalar(
                            out=ha1[:, mc, qoff:qoff + qn], in0=ps[:],
                            scalar1=bl(0, 3, mc), scalar2=None, op0=ALU.add)
                        nc.scalar.activation(sq1[:, mc, qoff:qoff + qn],
                                             ps[:], ACT.Square,
                                             bias=bl(0, 3, mc))
                layer_norm_T(0, ha1, sq1, h1, KEPT, PCH, pp3)

            # ================= LAYER 2 =================
            k2T = wp.tile([128, 2, KEPT], BF16, tag="k2T")
            v2N = wp.tile([128, KT, H, 65], BF16, tag="v2N")
            expb2 = wp.tile([128, H * KT], f32, tag="expb2")
            nc.scalar.activation(expb2[:], b2[:], ACT.Exp)
            q2 = wp.tile([128, 2, 1], BF16, tag="q2")
            bvb2 = wp.tile([128, D], f32, tag="bvb2")
            nc.gpsimd.partition_broadcast(bvb2[:], bvrow[0:1, 1, :])
            with tc.tile_pool(name="psP4", bufs=2, space="PSUM") as pp4:
                proj_T(1, 1, h1, PCH, k2T, pp4, "pk2")
                proj_V(1, h1, v2N, bvb2, expb2, pp4, "pv2")
                for mc in range(2):
                    ps = pp4.tile([128, 1], f32, tag="pq2")
                    for kc in range(2):
                        nc.tensor.matmul(ps[:], Wl(1, 0, kc, mc),
                                         h1[:, kc, 0:1],
                                         start=(kc == 0), stop=(kc == 1))
                    nc.vector.tensor_scalar(out=q2[:, mc, :], in0=ps[:],
                                            scalar1=bl(1, 0, mc),
                                            scalar2=None, op0=ALU.add)

            ctx2T = wp.tile([128, 2, 1], BF16, tag="ctx2T")
            exp2 = wp.tile([128, H, KT], BF16, tag="exp2")
            with tc.tile_pool(name="psP5", bufs=2, space="PSUM") as pp5:
                for hp in range(2):
                    mc = hp
                    for hh in range(2):
                        h = hp * 2 + hh
                        hr = slice(hh * 64, hh * 64 + 64)
                        s2_ps = pp5.tile([128, KT], f32, tag="s2")
                        for kt in range(KT):
                            nc.tensor.matmul(
                                s2_ps[:, kt:kt + 1],
                                k2T[hr, mc, kt * 128:(kt + 1) * 128],
                                q2[hr, mc, :], start=True, stop=True,
                                tile_position=(hh * 64, 0))
                        nc.scalar.activation(exp2[:, h, :], s2_ps[:],
                                             ACT.Exp, scale=float(SCALE))
                        c2_ps = pp5.tile([128, 1], f32, tag="c2")
                        for kt in range(KT):
                            nc.tensor.matmul(
                                c2_ps[0:65, :],
                                v2N[:, kt, h, :],
                                exp2[:, h, kt:kt + 1],
                                start=(kt == 0), stop=(kt == KT - 1))
                        r2 = rp.tile([1, 1], f32, tag="r2", bufs=2)
                        nc.vector.reciprocal(r2[:], c2_ps[64:65, :])
                        r2b = tp.tile([64, 1], f32, tag="r2b")
                        nc.gpsimd.partition_broadcast(r2b[:], r2[:])
                        nc.vector.tensor_tensor(
                            out=ctx2T[hh * 64:hh * 64 + 64, hp, :],
                            in0=c2_ps[0:64, :], in1=r2b[:], op=ALU.mult)

            h2 = wp.tile([128, 2, 1], F32R, tag="h2")
            sq2 = wp.tile([128, 2, 1], F32R, tag="sq2")
            h2n = wp.tile([128, 2, 1], BF16, tag="h2n")
            with tc.tile_pool(name="psP6", bufs=1, space="PSUM") as pp6:
                for mc in range(2):
                    ps = pp6.tile([128, 1], f32, tag="wo2", bufs=2)
                    for kc in range(2):
                        nc.tensor.matmul(ps[:], Wl(1, 3, kc, mc),
                                         ctx2T[:, kc, :],
                                         start=(kc == 0), stop=(kc == 1))
                    nc.vector.tensor_scalar(
                        out=h2[:, mc, :], in0=ps[:],
                        scalar1=bl(1, 3, mc), scalar2=None, op0=ALU.add)
                    nc.scalar.activation(sq2[:, mc, :], ps[:], ACT.Square,
                                         bias=bl(1, 3, mc))
                layer_norm_T(1, h2, sq2, h2n, 1, [(0, 1)], pp6)

                # classifier
                hid_ps = pp6.tile([128, 1], f32, tag="hid")
                for kc in range(2):
                    nc.tensor.matmul(hid_ps[:],
                                     wc1[:, kc * 128:(kc + 1) * 128],
                                     h2n[:, kc, :],
                                     start=(kc == 0), stop=(kc == 1))
                hid = wp.tile([128, 1], BF16, tag="hid_sb")
                nc.scalar.activation(hid[:], hid_ps[:], ACT.Relu,
                                     bias=bc1[:, 0:1])
                z_ps = pp6.tile([1, 1], f32, tag="z")
                nc.tensor.matmul(z_ps[:], wc2[:], hid[:],
                                 start=True, stop=True)
                nbc2 = rp.tile([1, 1], f32, tag="nbc2")
                nc.vector.tensor_scalar(out=nbc2[:], in0=bc2[:], scalar1=-1.0,
                                        scalar2=None, op0=ALU.mult)
                ez = rp.tile([1, 1], f32, tag="ez")
                nc.scalar.activation(ez[:], z_ps[:], ACT.Exp, scale=-1.0,
                                     bias=nbc2[:])
                den = rp.tile([1, 1], f32, tag="den")
                nc.vector.tensor_scalar(out=den[:], in0=ez[:], scalar1=1.0,
                                        scalar2=None, op0=ALU.add)
                sig = rp.tile([1, 1], f32, tag="sig")
                nc.vector.reciprocal(sig[:], den[:])
                nc.sync.dma_start(out_d[:], sig[:])

    nc.compile()
    return nc


def _get_nc():
    if "nc" not in _CACHE:
        _CACHE["nc"] = _build()
    return _CACHE["nc"]


def _chunk2(a):
    """[D, N] -> [128, 2, N] splitting dim0 into 2 partition chunks."""
    n = a.shape[1]
    return np.ascontiguousarray(
        a.reshape(2, 128, n).transpose(1, 0, 2), dtype=np.float32)


def _host_prep(x, time_deltas, mask, event_emb, time_emb, Wq, bq, Wk, bk,
               Wv, bv, time_proj, Wo, bo, ln_g, ln_b, Wc1, bc1, Wc2, bc2):
    import ml_dtypes
    bf16 = ml_dtypes.bfloat16
    x = np.asarray(x, np.int64)
    tb = np.clip(np.asarray(time_deltas, np.int64), 0, T - 1)
    mask = np.asarray(mask, np.int64)
    event_emb = np.asarray(event_emb, np.float32)
    time_emb = np.asarray(time_emb, np.float32)
    time_proj = np.asarray(time_proj, np.float32)

    # weights (identical on every core)
    wall = np.zeros((128, LAYERS * 4 * 2 * D), np.float32)
    ball = np.zeros((128, LAYERS * 4 * 2), np.float32)
    projs = [(Wq, bq), (Wk, bk), (Wv, bv), (Wo, bo)]
    for l in range(LAYERS):
        for p, (W, b) in enumerate(projs):
            Wmat = np.asarray(W[l], np.float32)  # [D, D] din x dout
            ch = Wmat.reshape(2, 128, D).transpose(1, 0, 2)  # [128, kc, dout]
            base = (l * 4 + p) * 2 * D
            wall[:, base:base + 2 * D] = ch.reshape(128, 2 * D)
            bb = np.asarray(b[l], np.float32).reshape(2, 128).T  # [128, kc]
            ball[:, (l * 4 + p) * 2:(l * 4 + p) * 2 + 2] = bb
    bvrow = np.stack([np.asarray(bv[l], np.float32) for l in range(LAYERS)])
    bvrow = bvrow.reshape(1, LAYERS, D)
    lngb = np.zeros((128, LAYERS * 2 * 2), np.float32)
    for l in range(LAYERS):
        for g, arr in enumerate([ln_g[l], ln_b[l]]):
            aa = np.asarray(arr, np.float32).reshape(2, 128).T
            lngb[:, (l * 2 + g) * 2:(l * 2 + g) * 2 + 2] = aa
    wc1 = np.asarray(Wc1, np.float32).reshape(2, 128, 128).transpose(
        1, 0, 2).reshape(128, 256)
    wc1 = np.ascontiguousarray(wc1)
    bc1a = np.asarray(bc1, np.float32).reshape(128, 1)
    wc2a = np.asarray(Wc2, np.float32).reshape(128, 1)
    bc2a = np.asarray(bc2, np.float32).reshape(1, 1)

    shared = {"wall": wall.astype(bf16), "ball": ball, "bvrow": bvrow,
              "lngb": lngb, "wc1": wc1.astype(bf16), "bc1": bc1a,
              "wc2": wc2a.astype(bf16), "bc2": bc2a,
              "onec": np.ones((128, 1), np.float32),
              "oneb": np.ones((128, 1), bf16)}

    in_maps = []
    for b_i in range(B):
        m = mask[b_i]
        last = S - 1
        idx = np.arange(S)
        unm = idx[(m != 0) & (idx != last)]
        assert 1 + len(unm) <= KEPT, f"kept overflow: {1 + len(unm)} > {KEPT}"
        order = np.concatenate(
            [[last], unm, idx[(m == 0) & (idx != last)]])[:KEPT]

        he_dev = _chunk2(event_emb[x[b_i][order]].T).astype(bf16)
        ht_dev = _chunk2(time_emb[tb[b_i][order]].T).astype(bf16)
        maskpen = np.where(m[order] == 0, np.float32(NEG), np.float32(0.0))

        def bias_dev(l):
            bias = time_proj[l][tb[b_i][order]] + maskpen[:, None]  # [KEPT,H]
            bb = bias.reshape(KT, 128, H).transpose(1, 2, 0)  # [p, h, kt]
            return np.ascontiguousarray(bb.reshape(128, H * KT), np.float32)

        core_map = {"he": he_dev, "ht": ht_dev, "b1": bias_dev(0),
                    "b2": bias_dev(1), **shared}
        in_maps.append(core_map)
        in_maps.append(core_map)
    return in_maps


def kernel(**inputs):
    from concourse.bass_utils import run_bass_kernel_spmd
    nc = _get_nc()
    in_maps = _host_prep(**inputs)
    res = run_bass_kernel_spmd(nc, in_maps, list(range(N_CORES)))
    out = np.zeros((B, 1), np.float32)
    for b_i in range(B):
        out[b_i, 0] = res.results[2 * b_i]["out"][0, 0]
    return out


# revision 28
# speedup vs baseline: 1.1634x; 1.1634x over previous
"""ChronoFormer Trainium2 kernel.

Sharding: batch-parallel, core pairs (2b, 2b+1) redundantly compute batch b
(no collectives — a pairwise AllGather measured 94us of dead link time, more
than the compute it saved). Host side does indexing only: per batch the
sequence is permuted to [orig pos 2047, unmasked keys..., masked...] and
truncated to KEPT=1280 slots. Masked keys contribute exp(-1e9)=0 via the
per-key bias, and layer outputs are only ever read at unmasked slots +
slot 0 (layer 2 needs keys at unmasked slots and the single query at orig
pos 2047), so the whole model runs on the KEPT range.

On-device: activations transposed (feature dim on partitions), bf16 matmul
operands with fp32 PSUM accumulation. Scores are computed transposed (keys
on partitions) so the per-key time-bias + mask and the 1/sqrt(dk) scale fold
into the scalar-engine exp (out = exp(scale*in + bias)); the two 512-wide
query chunks of one (key-tile, head) share that bias, so their exp runs as
one strided ACT call over a 2-bank PSUM tile. Softmax stays unnormalized
through attn@V via a ones-column augmentation of V (row 64 of the ctx PSUM
accumulates the denominator); normalization is applied to the small ctx.
LayerNorm: ones-vector matmuls (fp32r) for partition sums, inv-std as
exp(-0.5*ln(var+eps)) — the whole kernel stays on one ACT table set
(natural_log_exp_and_others); the final sigmoid is exp + DVE reciprocal.
"""

import numpy as np

B, S, D, H, DK, LAYERS = 4, 2048, 256, 4, 64, 2
V, T = 32000, 1000
KEPT = 1280
KT = KEPT // 128          # 10 key tiles
PCH = [(0, 512), (512, 512), (1024, 256)]  # kept-range free-dim chunks
N_CORES = 8
EPS = 1e-5
SCALE = 1.0 / np.sqrt(DK).astype(np.float32)
NEG = -1e9

_CACHE = {}


def _build():
    import concourse.bass as bass
    import concourse.mybir as mybir
    import concourse.tile as tile
    from concourse import bacc

    f32 = mybir.dt.float32
    F32R = mybir.dt.float32r
    BF16 = mybir.dt.bfloat16
    ACT = mybir.ActivationFunctionType
    ALU = mybir.AluOpType

    nc = bacc.Bacc("TRN2", target_bir_lowering=False, debug=True,
                   num_devices=N_CORES)

    he_d = nc.dram_tensor("he", [128, 2, KEPT], BF16, kind="ExternalInput")
    ht_d = nc.dram_tensor("ht", [128, 2, KEPT], BF16, kind="ExternalInput")
    b1_d = nc.dram_tensor("b1", [128, H * KT], f32, kind="ExternalInput")
    b2_d = nc.dram_tensor("b2", [128, H * KT], f32, kind="ExternalInput")
    wall_d = nc.dram_tensor("wall", [128, LAYERS * 4 * 2 * D], BF16,
                            kind="ExternalInput")
    ball_d = nc.dram_tensor("ball", [128, LAYERS * 4 * 2], f32,
                            kind="ExternalInput")
    bvrow_d = nc.dram_tensor("bvrow", [1, LAYERS, D], f32, kind="ExternalInput")
    lngb_d = nc.dram_tensor("lngb", [128, LAYERS * 2 * 2], f32,
                            kind="ExternalInput")
    wc1_d = nc.dram_tensor("wc1", [128, 2 * 128], BF16, kind="ExternalInput")
    bc1_d = nc.dram_tensor("bc1", [128, 1], f32, kind="ExternalInput")
    wc2_d = nc.dram_tensor("wc2", [128, 1], BF16, kind="ExternalInput")
    bc2_d = nc.dram_tensor("bc2", [1, 1], f32, kind="ExternalInput")
    onec_d = nc.dram_tensor("onec", [128, 1], F32R, kind="ExternalInput")
    oneb_d = nc.dram_tensor("oneb", [128, 1], BF16, kind="ExternalInput")
    out_d = nc.dram_tensor("out", [1, 1], f32, kind="ExternalOutput")

    def r(ap):
        return ap.bitcast(F32R)

    with tile.TileContext(nc) as tc:
        with (
            tc.tile_pool(name="const", bufs=1) as cp,
            tc.tile_pool(name="work", bufs=1) as wp,
            tc.tile_pool(name="exp", bufs=4) as ep,
            tc.tile_pool(name="tmp", bufs=2) as tp,
            tc.tile_pool(name="rows", bufs=1) as rp,
        ):
            # ---- constant / input loads ----
            wall = cp.tile([128, LAYERS * 4 * 2 * D], BF16, tag="wall")
            nc.sync.dma_start(wall[:], wall_d[:])
            ball = cp.tile([128, LAYERS * 4 * 2], f32, tag="ball")
            nc.sync.dma_start(ball[:], ball_d[:])
            bvrow = cp.tile([1, LAYERS, D], f32, tag="bvrow")
            nc.sync.dma_start(bvrow[:], bvrow_d[:])
            lngb = cp.tile([128, LAYERS * 2 * 2], f32, tag="lngb")
            nc.sync.dma_start(lngb[:], lngb_d[:])
            b1 = cp.tile([128, H * KT], f32, tag="b1")
            nc.sync.dma_start(b1[:], b1_d[:])
            b2 = cp.tile([128, H * KT], f32, tag="b2")
            nc.sync.dma_start(b2[:], b2_d[:])
            wc1 = cp.tile([128, 2 * 128], BF16, tag="wc1")
            nc.sync.dma_start(wc1[:], wc1_d[:])
            bc1 = cp.tile([128, 1], f32, tag="bc1")
            nc.sync.dma_start(bc1[:], bc1_d[:])
            wc2 = cp.tile([128, 1], BF16, tag="wc2")
            nc.sync.dma_start(wc2[:], wc2_d[:])
            bc2 = cp.tile([1, 1], f32, tag="bc2")
            nc.sync.dma_start(bc2[:], bc2_d[:])
            ones_col = cp.tile([128, 1], F32R, tag="ones")
            nc.sync.dma_start(ones_col[:], onec_d[:])
            oneb = cp.tile([128, 1], BF16, tag="oneb")
            nc.sync.dma_start(oneb[:], oneb_d[:])
            eps_t = cp.tile([1, 1], f32, tag="eps")
            nc.vector.memset(eps_t[:], EPS)

            # PE warm-up burst + early ACT table load: runs during the input
            # DMA wait so HAM un-throttles and the exp table set is resident
            # before the first real matmul/activation.
            with tc.tile_pool(name="warm", bufs=1, space="PSUM") as wpp:
                wps = wpp.tile([128, 512], f32, tag="warm")
                wsb = cp.tile([128, 512], BF16, tag="wsb")
                nc.vector.tensor_copy(wsb[:],
                                      oneb[:, 0:1].broadcast_to([128, 512]))
                wout = cp.tile([1, 1], f32, tag="wout")
                for wi in range(12):
                    nc.tensor.matmul(wps[0:1, :], oneb[:], wsb[:],
                                     start=True, stop=True)
                nc.scalar.activation(wout[:], eps_t[:], ACT.Exp)

            he = wp.tile([128, 2, KEPT], BF16, tag="he")
            nc.sync.dma_start(he[:], he_d[:])
            ht = wp.tile([128, 2, KEPT], BF16, tag="ht")
            nc.sync.dma_start(ht[:], ht_d[:])

            def Wl(l, p, kc, mc):
                base = (((l * 4 + p) * 2 + kc) * D) + mc * 128
                return wall[:, base:base + 128]

            def Wfull(l, p, kc):
                base = ((l * 4 + p) * 2 + kc) * D
                return wall[:, base:base + D]

            def bl(l, p, mc):
                c = (l * 4 + p) * 2 + mc
                return ball[:, c:c + 1]

            def gb(l, g, kc):
                c = (l * 2 + g) * 2 + kc
                return lngb[:, c:c + 1]

            # embedding add (rounds to bf16 on write)
            h0 = wp.tile([128, 2, KEPT], BF16, tag="h0")
            nc.vector.tensor_tensor(out=h0[:], in0=he[:], in1=ht[:],
                                    op=ALU.add)

            def proj_T(l, p, rhs, chunks, out_sb, psum_pool, ptag):
                # transposed-output projection: out[dout, s] over given chunks
                for mc in range(2):
                    for (off, n) in chunks:
                        ps = psum_pool.tile([128, n], f32, tag=ptag)
                        for kc in range(2):
                            nc.tensor.matmul(
                                ps[:], Wl(l, p, kc, mc),
                                rhs[:, kc, off:off + n],
                                start=(kc == 0), stop=(kc == 1))
                        nc.vector.tensor_scalar(
                            out=out_sb[:, mc, off:off + n], in0=ps[:],
                            scalar1=bl(l, p, mc), scalar2=None, op0=ALU.add)

            def proj_V(l, rhs, out_sb, bvb, psum_pool, ptag):
                # natural-output V projection into [s_tile, h, 0:64];
                # col 64 holds the ones column (softmax denominator trick)
                for st in range(KT):
                    ps = psum_pool.tile([128, D], f32, tag=ptag)
                    for kc in range(2):
                        nc.tensor.matmul(
                            ps[:], rhs[:, kc, st * 128:(st + 1) * 128],
                            Wfull(l, 2, kc),
                            start=(kc == 0), stop=(kc == 1))
                    nc.vector.tensor_tensor(
                        out=out_sb[:, st, :, 0:64],
                        in0=ps[:].rearrange("p (h d) -> p h d", d=64),
                        in1=bvb[:].rearrange("p (h d) -> p h d", d=64),
                        op=ALU.add)

            def layer_norm_T(l, ha, sq, out_sb, ncols, chunks, stat_pool):
                # stats + apply; ha/sq [128, 2, ncols] f32r SBUF
                m_row = rp.tile([1, ncols], f32, tag=f"m{l}")
                v_row = rp.tile([1, ncols], f32, tag=f"v{l}")
                for (off, n) in chunks:
                    ssum = stat_pool.tile([1, n], f32, tag="sts")
                    ssq = stat_pool.tile([1, n], f32, tag="stq")
                    g_ = r if n >= 256 else (lambda a: a.bitcast(f32))
                    for kc in range(2):
                        nc.tensor.matmul(ssum[:], g_(ones_col[:]),
                                         g_(ha[:, kc, off:off + n]),
                                         start=(kc == 0), stop=(kc == 1))
                        nc.tensor.matmul(ssq[:], g_(ones_col[:]),
                                         g_(sq[:, kc, off:off + n]),
                                         start=(kc == 0), stop=(kc == 1))
                    nc.vector.tensor_scalar(out=m_row[:, off:off + n],
                                            in0=ssum[:], scalar1=1.0 / D,
                                            scalar2=None, op0=ALU.mult)
                    nc.vector.tensor_scalar(out=v_row[:, off:off + n],
                                            in0=ssq[:], scalar1=1.0 / D,
                                            scalar2=None, op0=ALU.mult)
                msq = rp.tile([1, ncols], f32, tag=f"msq{l}")
                nc.vector.tensor_tensor(out=msq[:], in0=m_row[:], in1=m_row[:],
                                        op=ALU.mult)
                nc.vector.tensor_tensor(out=v_row[:], in0=v_row[:], in1=msq[:],
                                        op=ALU.subtract)
                vln = rp.tile([1, ncols], f32, tag=f"vln{l}")
                nc.scalar.activation(vln[:], v_row[:], ACT.Ln,
                                     bias=eps_t[0:1, 0:1])
                inv_row = rp.tile([1, ncols], f32, tag=f"inv{l}")
                nc.scalar.activation(inv_row[:], vln[:], ACT.Exp, scale=-0.5)
                m2_row = rp.tile([1, ncols], f32, tag=f"m2{l}")
                nc.vector.scalar_tensor_tensor(
                    out=m2_row[:], in0=m_row[:], scalar=-1.0, in1=inv_row[:],
                    op0=ALU.mult, op1=ALU.mult)
                invb = tp.tile([128, ncols], f32, tag="invb")
                nc.gpsimd.partition_broadcast(invb[:], inv_row[:])
                m2b = tp.tile([128, ncols], f32, tag="m2b")
                nc.gpsimd.partition_broadcast(m2b[:], m2_row[:])
                for kc in range(2):
                    for (off, n) in chunks:
                        t1 = tp.tile([128, n], f32, tag="t1")
                        nc.vector.tensor_tensor(
                            out=t1[:], in0=ha[:, kc, off:off + n],
                            in1=invb[:, off:off + n], op=ALU.mult)
                        t2 = tp.tile([128, n], f32, tag="t2")
                        nc.vector.tensor_tensor(
                            out=t2[:], in0=t1[:], in1=m2b[:, off:off + n],
                            op=ALU.add)
                        nc.vector.tensor_scalar(
                            out=out_sb[:, kc, off:off + n], in0=t2[:],
                            scalar1=gb(l, 0, kc), scalar2=gb(l, 1, kc),
                            op0=ALU.mult, op1=ALU.add)

            def attn_layer(qT, bias_t, kT, vN, ctxT, hp):
                # scoresT -> exp (no bias: folded into V) -> ctx+denominator.
                # Main pass: query chunks (0,512),(512,512) in one 2-bank
                # PSUM tile per (kt, head), one strided exp call; the two
                # heads of chunk hp are emitted adjacently as row-groups
                # (0,*) / (64,*) so their score matmuls run concurrently.
                mc = hp
                pool = tc.tile_pool(name=f"psA{hp}", bufs=1, space="PSUM")
                psum_pool = pool.__enter__()
                ctx_ps = {}
                for hh in range(2):
                    for j in range(2):
                        ctx_ps[(hh, j)] = psum_pool.tile(
                            [65, 512], f32, name=f"ctx{hh}{j}",
                            tag=f"ctx{hh}{j}", bufs=1)
                for kt in range(KT):
                    s_t = {}
                    for hh in range(2):
                        s_t[hh] = psum_pool.tile([128, 1024], f32, tag="s",
                                                 name="s", bufs=2)
                    for j in range(2):
                        for hh in range(2):
                            hr = slice(hh * 64, hh * 64 + 64)
                            nc.tensor.matmul(
                                s_t[hh][:, j * 512:(j + 1) * 512],
                                kT[hr, mc, kt * 128:(kt + 1) * 128],
                                qT[hr, mc, j * 512:(j + 1) * 512],
                                start=True, stop=True,
                                tile_position=(hh * 64, 0))
                    for hh in range(2):
                        h = hp * 2 + hh
                        e_sb = ep.tile([128, 2, 512], BF16, tag="e")
                        nc.scalar.activation(
                            e_sb[:],
                            s_t[hh][:].rearrange("p (c q) -> p c q", c=2),
                            ACT.Exp, scale=float(SCALE),
                            bias=bias_t[:, h * KT + kt:h * KT + kt + 1])
                        for j in range(2):
                            nc.tensor.matmul(
                                ctx_ps[(hh, j)][0:65, :],
                                vN[:, kt, h, :], e_sb[:, j, :],
                                start=(kt == 0), stop=(kt == KT - 1))
                for hh in range(2):
                    for j in range(2):
                        _ctx_norm(ctx_ps[(hh, j)], ctxT, hh, hp, j * 512, 512)
                pool.__exit__(None, None, None)
                # tail pass: queries 1024:1280, both heads in one PSUM bank
                pool = tc.tile_pool(name=f"psAt{hp}", bufs=1, space="PSUM")
                tail_pool = pool.__enter__()
                ctx_tl = {}
                for hh in range(2):
                    ctx_tl[hh] = tail_pool.tile([65, 256], f32,
                                                name=f"ctxt{hh}",
                                                tag=f"ctxt{hh}", bufs=1)
                for kt in range(KT):
                    s_t = {}
                    for hh in range(2):
                        s_t[hh] = tail_pool.tile([128, 256], f32,
                                                 name="st", tag=f"st{hh}",
                                                 bufs=2)
                        hr = slice(hh * 64, hh * 64 + 64)
                        nc.tensor.matmul(
                            s_t[hh][:],
                            kT[hr, mc, kt * 128:(kt + 1) * 128],
                            qT[hr, mc, 1024:1280],
                            start=True, stop=True,
                            tile_position=(hh * 64, 0))
                    for hh in range(2):
                        h = hp * 2 + hh
                        e_sb = ep.tile([128, 256], BF16, tag="et")
                        nc.scalar.activation(
                            e_sb[:], s_t[hh][:], ACT.Exp, scale=float(SCALE),
                            bias=bias_t[:, h * KT + kt:h * KT + kt + 1])
                        nc.tensor.matmul(
                            ctx_tl[hh][0:65, :],
                            vN[:, kt, h, :], e_sb[:],
                            start=(kt == 0), stop=(kt == KT - 1))
                for hh in range(2):
                    _ctx_norm(ctx_tl[hh], ctxT, hh, hp, 1024, 256)
                pool.__exit__(None, None, None)

            def _ctx_norm(ctx_ps, ctxT, hh, hp, qo, qn):
                r_sb = rp.tile([1, qn], f32, tag="r1", bufs=2, name="r_sb")
                nc.vector.reciprocal(r_sb[:], ctx_ps[64:65, :])
                rb = tp.tile([64, qn], f32, tag="rb", name="rb")
                nc.gpsimd.partition_broadcast(rb[:], r_sb[:])
                nc.vector.tensor_tensor(
                    out=ctxT[hh * 64:hh * 64 + 64, hp, qo:qo + qn],
                    in0=ctx_ps[0:64, :], in1=rb[:], op=ALU.mult)

            # ================= LAYER 1 =================
            kT1 = wp.tile([128, 2, KEPT], BF16, tag="kT1")
            qT1 = wp.tile([128, 2, KEPT], BF16, tag="qT1")
            vN1 = wp.tile([128, KT, H, 65], BF16, tag="vN1")
            nc.vector.tensor_copy(
                vN1[:, :, :, 64:65],
                oneb[:, 0:1].unsqueeze(1).broadcast_to([128, KT, H, 1]))
            bvb1 = wp.tile([128, D], f32, tag="bvb1")
            nc.gpsimd.partition_broadcast(bvb1[:], bvrow[0:1, 0, :])

            with tc.tile_pool(name="psP1", bufs=2, space="PSUM") as pp1:
                proj_T(0, 1, h0, PCH, kT1, pp1, "pk")
                proj_T(0, 0, h0, PCH, qT1, pp1, "pq")
                proj_V(0, h0, vN1, bvb1, pp1, "pv")

            ctxT1 = wp.tile([128, 2, KEPT], BF16, tag="ctxT1")
            for hp in range(2):
                attn_layer(qT1, b1, kT1, vN1, ctxT1, hp)

            ha1 = wp.tile([128, 2, KEPT], F32R, tag="ha1")
            sq1 = wp.tile([128, 2, KEPT], F32R, tag="sq1")
            h1 = wp.tile([128, 2, KEPT], BF16, tag="h1")
            with tc.tile_pool(name="psP3", bufs=2, space="PSUM") as pp3:
                for mc in range(2):
                    for (qoff, qn) in PCH:
                        ps = pp3.tile([128, qn], f32, tag="wo")
                        for kc in range(2):
                            nc.tensor.matmul(ps[:], Wl(0, 3, kc, mc),
                                             ctxT1[:, kc, qoff:qoff + qn],
                                             start=(kc == 0), stop=(kc == 1))
                        nc.vector.tensor_scalar(
                            out=ha1[:, mc, qoff:qoff + qn], in0=ps[:],
                            scalar1=bl(0, 3, mc), scalar2=None, op0=ALU.add)
                        nc.scalar.activation(sq1[:, mc, qoff:qoff + qn],
                                             ps[:], ACT.Square,
                                             bias=bl(0, 3, mc))
                layer_norm_T(0, ha1, sq1, h1, KEPT, PCH, pp3)

            # ================= LAYER 2 =================
            k2T = wp.tile([128, 2, KEPT], BF16, tag="k2T")
            v2N = wp.tile([128, KT, H, 65], BF16, tag="v2N")
            nc.vector.tensor_copy(
                v2N[:, :, :, 64:65],
                oneb[:, 0:1].unsqueeze(1).broadcast_to([128, KT, H, 1]))
            q2 = wp.tile([128, 2, 1], BF16, tag="q2")
            bvb2 = wp.tile([128, D], f32, tag="bvb2")
            nc.gpsimd.partition_broadcast(bvb2[:], bvrow[0:1, 1, :])
            with tc.tile_pool(name="psP4", bufs=2, space="PSUM") as pp4:
                proj_T(1, 1, h1, PCH, k2T, pp4, "pk2")
                proj_V(1, h1, v2N, bvb2, pp4, "pv2")
                for mc in range(2):
                    ps = pp4.tile([128, 1], f32, tag="pq2")
                    for kc in range(2):
                        nc.tensor.matmul(ps[:], Wl(1, 0, kc, mc),
                                         h1[:, kc, 0:1],
                                         start=(kc == 0), stop=(kc == 1))
                    nc.vector.tensor_scalar(out=q2[:, mc, :], in0=ps[:],
                                            scalar1=bl(1, 0, mc),
                                            scalar2=None, op0=ALU.add)

            ctx2T = wp.tile([128, 2, 1], BF16, tag="ctx2T")
            exp2 = wp.tile([128, H, KT], BF16, tag="exp2")
            with tc.tile_pool(name="psP5", bufs=2, space="PSUM") as pp5:
                for hp in range(2):
                    mc = hp
                    for hh in range(2):
                        h = hp * 2 + hh
                        hr = slice(hh * 64, hh * 64 + 64)
                        s2_ps = pp5.tile([128, KT], f32, tag="s2")
                        for kt in range(KT):
                            nc.tensor.matmul(
                                s2_ps[:, kt:kt + 1],
                                k2T[hr, mc, kt * 128:(kt + 1) * 128],
                                q2[hr, mc, :], start=True, stop=True,
                                tile_position=(hh * 64, 0))
                        s2e = tp.tile([128, KT], f32, tag="s2e")
                        nc.vector.scalar_tensor_tensor(
                            out=s2e[:], in0=s2_ps[:], scalar=float(SCALE),
                            in1=b2[:, h * KT:(h + 1) * KT],
                            op0=ALU.mult, op1=ALU.add)
                        nc.scalar.activation(exp2[:, h, :], s2e[:], ACT.Exp)
                        c2_ps = pp5.tile([128, 1], f32, tag="c2")
                        for kt in range(KT):
                            nc.tensor.matmul(
                                c2_ps[0:65, :],
                                v2N[:, kt, h, :],
                                exp2[:, h, kt:kt + 1],
                                start=(kt == 0), stop=(kt == KT - 1))
                        r2 = rp.tile([1, 1], f32, tag="r2", bufs=2)
                        nc.vector.reciprocal(r2[:], c2_ps[64:65, :])
                        r2b = tp.tile([64, 1], f32, tag="r2b")
                        nc.gpsimd.partition_broadcast(r2b[:], r2[:])
                        nc.vector.tensor_tensor(
                            out=ctx2T[hh * 64:hh * 64 + 64, hp, :],
                            in0=c2_ps[0:64, :], in1=r2b[:], op=ALU.mult)

            h2 = wp.tile([128, 2, 1], F32R, tag="h2")
            sq2 = wp.tile([128, 2, 1], F32R, tag="sq2")
            h2n = wp.tile([128, 2, 1], BF16, tag="h2n")
            with tc.tile_pool(name="psP6", bufs=1, space="PSUM") as pp6:
                for mc in range(2):
                    ps = pp6.tile([128, 1], f32, tag="wo2", bufs=2)
                    for kc in range(2):
                        nc.tensor.matmul(ps[:], Wl(1, 3, kc, mc),
                                         ctx2T[:, kc, :],
                                         start=(kc == 0), stop=(kc == 1))
                    nc.vector.tensor_scalar(
                        out=h2[:, mc, :], in0=ps[:],
                        scalar1=bl(1, 3, mc), scalar2=None, op0=ALU.add)
                    nc.scalar.activation(sq2[:, mc, :], ps[:], ACT.Square,
                                         bias=bl(1, 3, mc))
                layer_norm_T(1, h2, sq2, h2n, 1, [(0, 1)], pp6)

                # classifier
                hid_ps = pp6.tile([128, 1], f32, tag="hid")
                for kc in range(2):
                    nc.tensor.matmul(hid_ps[:],
                                     wc1[:, kc * 128:(kc + 1) * 128],
                                     h2n[:, kc, :],
                                     start=(kc == 0), stop=(kc == 1))
                hid = wp.tile([128, 1], BF16, tag="hid_sb")
                nc.scalar.activation(hid[:], hid_ps[:], ACT.Relu,
                                     bias=bc1[:, 0:1])
                z_ps = pp6.tile([1, 1], f32, tag="z")
                nc.tensor.matmul(z_ps[:], wc2[:], hid[:],
                                 start=True, stop=True)
                nbc2 = rp.tile([1, 1], f32, tag="nbc2")
                nc.vector.tensor_scalar(out=nbc2[:], in0=bc2[:], scalar1=-1.0,
                                        scalar2=None, op0=ALU.mult)
                ez = rp.tile([1, 1], f32, tag="ez")
                nc.scalar.activation(ez[:], z_ps[:], ACT.Exp, scale=-1.0,
                                     bias=nbc2[:])
                den = rp.tile([1, 1], f32, tag="den")
                nc.vector.tensor_scalar(out=den[:], in0=ez[:], scalar1=1.0,
                                        scalar2=None, op0=ALU.add)
                sig = rp.tile([1, 1], f32, tag="sig")
                nc.vector.reciprocal(sig[:], den[:])
                nc.sync.dma_start(out_d[:], sig[:])

    nc.compile()
    return nc


def _get_nc():
    if "nc" not in _CACHE:
        _CACHE["nc"] = _build()
    return _CACHE["nc"]


def _chunk2(a):
    """[D, N] -> [128, 2, N] splitting dim0 into 2 partition chunks."""
    n = a.shape[1]
    return np.ascontiguousarray(
        a.reshape(2, 128, n).transpose(1, 0, 2), dtype=np.float32)


def _host_prep(x, time_deltas, mask, event_emb, time_emb, Wq, bq, Wk, bk,
               Wv, bv, time_proj, Wo, bo, ln_g, ln_b, Wc1, bc1, Wc2, bc2):
    import ml_dtypes
    bf16 = ml_dtypes.bfloat16
    x = np.asarray(x, np.int64)
    tb = np.clip(np.asarray(time_deltas, np.int64), 0, T - 1)
    mask = np.asarray(mask, np.int64)
    event_emb = np.asarray(event_emb, np.float32)
    time_emb = np.asarray(time_emb, np.float32)
    time_proj = np.asarray(time_proj, np.float32)

    # weights (identical on every core)
    wall = np.zeros((128, LAYERS * 4 * 2 * D), np.float32)
    ball = np.zeros((128, LAYERS * 4 * 2), np.float32)
    projs = [(Wq, bq), (Wk, bk), (Wv, bv), (Wo, bo)]
    for l in range(LAYERS):
        for p, (W, b) in enumerate(projs):
            Wmat = np.asarray(W[l], np.float32)  # [D, D] din x dout
            ch = Wmat.reshape(2, 128, D).transpose(1, 0, 2)  # [128, kc, dout]
            base = (l * 4 + p) * 2 * D
            wall[:, base:base + 2 * D] = ch.reshape(128, 2 * D)
            bb = np.asarray(b[l], np.float32).reshape(2, 128).T  # [128, kc]
            ball[:, (l * 4 + p) * 2:(l * 4 + p) * 2 + 2] = bb
    bvrow = np.stack([np.asarray(bv[l], np.float32) for l in range(LAYERS)])
    bvrow = bvrow.reshape(1, LAYERS, D)
    lngb = np.zeros((128, LAYERS * 2 * 2), np.float32)
    for l in range(LAYERS):
        for g, arr in enumerate([ln_g[l], ln_b[l]]):
            aa = np.asarray(arr, np.float32).reshape(2, 128).T
            lngb[:, (l * 2 + g) * 2:(l * 2 + g) * 2 + 2] = aa
    wc1 = np.asarray(Wc1, np.float32).reshape(2, 128, 128).transpose(
        1, 0, 2).reshape(128, 256)
    wc1 = np.ascontiguousarray(wc1)
    bc1a = np.asarray(bc1, np.float32).reshape(128, 1)
    wc2a = np.asarray(Wc2, np.float32).reshape(128, 1)
    bc2a = np.asarray(bc2, np.float32).reshape(1, 1)

    shared = {"wall": wall.astype(bf16), "ball": ball, "bvrow": bvrow,
              "lngb": lngb, "wc1": wc1.astype(bf16), "bc1": bc1a,
              "wc2": wc2a.astype(bf16), "bc2": bc2a,
              "onec": np.ones((128, 1), np.float32),
              "oneb": np.ones((128, 1), bf16)}

    in_maps = []
    for b_i in range(B):
        m = mask[b_i]
        last = S - 1
        idx = np.arange(S)
        unm = idx[(m != 0) & (idx != last)]
        assert 1 + len(unm) <= KEPT, f"kept overflow: {1 + len(unm)} > {KEPT}"
        order = np.concatenate(
            [[last], unm, idx[(m == 0) & (idx != last)]])[:KEPT]

        he_dev = _chunk2(event_emb[x[b_i][order]].T).astype(bf16)
        ht_dev = _chunk2(time_emb[tb[b_i][order]].T).astype(bf16)
        maskpen = np.where(m[order] == 0, np.float32(NEG), np.float32(0.0))

        def bias_dev(l):
            bias = time_proj[l][tb[b_i][order]] + maskpen[:, None]  # [KEPT,H]
            bb = bias.reshape(KT, 128, H).transpose(1, 2, 0)  # [p, h, kt]
            return np.ascontiguousarray(bb.reshape(128, H * KT), np.float32)

        core_map = {"he": he_dev, "ht": ht_dev, "b1": bias_dev(0),
                    "b2": bias_dev(1), **shared}
        in_maps.append(core_map)
        in_maps.append(core_map)
    return in_maps


def kernel(**inputs):
    from concourse.bass_utils import run_bass_kernel_spmd
    nc = _get_nc()
    in_maps = _host_prep(**inputs)
    res = run_bass_kernel_spmd(nc, in_maps, list(range(N_CORES)))
    out = np.zeros((B, 1), np.float32)
    for b_i in range(B):
        out[b_i, 0] = res.results[2 * b_i]["out"][0, 0]
    return out
